# revision 1
# baseline (speedup 1.0000x reference)
"""GroupMixAttention Trainium2 kernel (8-core SPMD, batch-parallel).

Problem: x[16,256,32,32]; per group g (4 groups of 64 ch):
  Q/K/V = wq/wk/wv[g] @ xg   (xg = [64, 1024])
  scores = (Q^T K)/8 ; attn = softmax(scores, -1) ; out = V @ attn^T
then y = wo @ concat(out).

Sharding: data-parallel over batch, 2 batches per core, no collectives.

Layout strategy per (batch, group-pair):
  - x2 [128, 1024] holds two groups' channels (natural slicing of x).
  - Q2/K2 [128, 1024] computed with row+col tiled matmuls (two 64-row
    groups concurrently on the PE array).
  - scoresT[m, n] = K^T Q computed chunk-wise (m in 8 chunks of 128) with
    the two groups packed in PE row-halves; exp on the scalar engine
    (scale=1/8 folded in; softmax max-subtraction skipped — scores are
    O(5) so fp32 exp is safe).
  - V^T chunks [128(m), 64(d)] computed directly (lhsT = x chunks) with a
    ones column appended -> PV matmul lhsT [128, 65]: row 64 of the
    accumulated output is the softmax denominator.
  - E and V^T are cast to fp16 so the K=128 PV matmul is single-pass
    (fp32 at K=128 is two-pass/half-rate on the PE array).
  - normalize (deferred to batch end, off the PE critical path): denom rows
    staged at 32-aligned partitions -> one batched DVE reciprocal -> K=1
    ones-matmul broadcast into PSUM -> DVE multiply.
  - out_proj: wo^T chunks as lhsT over the stacked normalized heads.
  - mc loop is software-pipelined (scores issued 2 steps ahead of exp+PV)
    so the PE never stalls on the scalar engine's exp.
"""

import os
import sys

import numpy as np

for _p in ("/opt/trn_rl_repo", "/root/.axon_site/_ro/trn_rl_repo"):
    if os.path.isdir(_p) and _p not in sys.path:
        sys.path.insert(0, _p)

import concourse.bass as bass
import concourse.mybir as mybir
import concourse.tile as tile
from concourse import bacc
from concourse.bass_utils import run_bass_kernel_spmd

F32 = mybir.dt.float32
BF16 = mybir.dt.float16
EXP = mybir.ActivationFunctionType.Exp
N_CORES = 8
B_PER_CORE = 2  # 16 batches / 8 cores
NT = 1024  # H*W
GD = 64    # group dim
ts = bass.ts


def _build_program():
    nc = bacc.Bacc("TRN2", target_bir_lowering=False, debug=False,
                   num_devices=N_CORES)
    xs = nc.dram_tensor("xs", [B_PER_CORE, 2, 128, NT], F32,
                        kind="ExternalInput").ap()
    wqT = nc.dram_tensor("wqT", [2, 128, GD], F32, kind="ExternalInput").ap()
    wkT = nc.dram_tensor("wkT", [2, 128, GD], F32, kind="ExternalInput").ap()
    wvT = nc.dram_tensor("wvT", [2, 128, GD], F32, kind="ExternalInput").ap()
    woT = nc.dram_tensor("woT", [2, 128, 256], F32, kind="ExternalInput").ap()
    y = nc.dram_tensor("y", [B_PER_CORE, 256, NT], F32,
                       kind="ExternalOutput").ap()

    with tile.TileContext(nc) as tc:
        from contextlib import ExitStack
        with ExitStack() as ctx:
            const = ctx.enter_context(tc.tile_pool(name="const", bufs=1))
            xpool = ctx.enter_context(tc.tile_pool(name="xp", bufs=2))
            qk = ctx.enter_context(tc.tile_pool(name="qk", bufs=2))
            vtp = ctx.enter_context(tc.tile_pool(name="vt", bufs=2))
            ep = ctx.enter_context(tc.tile_pool(name="ep", bufs=3))
            sm = ctx.enter_context(tc.tile_pool(name="sm", bufs=2))
            onp = ctx.enter_context(tc.tile_pool(name="on", bufs=2))
            yp = ctx.enter_context(tc.tile_pool(name="yp", bufs=2))
            psS = ctx.enter_context(
                tc.tile_pool(name="psS", bufs=3, space="PSUM"))
            psAcc = ctx.enter_context(
                tc.tile_pool(name="psAcc", bufs=1, space="PSUM"))

            # Load weights once.
            w_sb = {}
            for name, dram in (("wq", wqT), ("wk", wkT), ("wv", wvT)):
                for p in range(2):
                    t = const.tile([128, GD], F32, tag=f"{name}{p}", name=f"{name}{p}")
                    nc.sync.dma_start(t[:], dram[p])
                    w_sb[name, p] = t
            ones128 = const.tile([128, 128], F32, tag="ones128",
                                 name="ones128")
            nc.gpsimd.memset(ones128[:], 1.0)
            wo_sb = []
            for k in range(2):
                t = const.tile([128, 256], F32, tag=f"wo{k}", name=f"wo{k}")
                nc.sync.dma_start(t[:], woT[k])
                wo_sb.append(t)

            for b in range(B_PER_CORE):
                outN = [onp.tile([128, NT], F32, tag=f"outN{p}", name=f"outN{p}")
                        for p in range(2)]
                norm_jobs = []
                for p in range(2):
                    x2 = xpool.tile([128, NT], F32, tag="x2")
                    nc.sync.dma_start(x2[:], xs[b, p])
                    den4 = sm.tile([128, 512], F32, tag="den4")

                    # K2 / Q2 projections, both groups packed on the array.
                    K2 = qk.tile([128, NT], F32, tag="K2")
                    Q2 = qk.tile([128, NT], F32, tag="Q2")
                    for wname, dst in (("wk", K2), ("wq", Q2)):
                        wt = w_sb[wname, p]
                        for nh in range(2):
                            s = ts(nh, 512)
                            ps = psS.tile([128, 512], F32, tag=f"pss{nh}",
                                          name=f"qkv{nh}")
                            nc.tensor.matmul(
                                ps[0:64, :], wt[0:64, :], x2[0:64, s],
                                start=True, stop=True, tile_position=(0, 0))
                            nc.tensor.matmul(
                                ps[64:128, :], wt[64:128, :], x2[64:128, s],
                                start=True, stop=True, tile_position=(64, 64))
                            nc.vector.tensor_copy(dst[:, s], ps[:])

                    # V^T chunks with ones column (denominator trick).
                    vts = [vtp.tile([128, 8 * (GD + 1)], BF16, tag=f"vt{g}", name=f"vt{g}")
                           for g in range(2)]
                    for g in range(2):
                        nc.vector.memset(vts[g][:], 1.0)
                    wv = w_sb["wv", p]
                    for mc in range(8):
                        pvA = psS.tile([128, GD], F32, tag="pss0")
                        pvB = psS.tile([128, GD], F32, tag="pss1")
                        nc.tensor.matmul(
                            pvA[:], x2[0:64, ts(mc, 128)], wv[0:64, :],
                            start=True, stop=True, tile_position=(0, 0))
                        nc.tensor.matmul(
                            pvB[:], x2[64:128, ts(mc, 128)], wv[64:128, :],
                            start=True, stop=True, tile_position=(64, 0))
                        c0 = 65 * mc
                        nc.vector.tensor_copy(vts[0][:, c0:c0 + GD], pvA[:])
                        nc.vector.tensor_copy(vts[1][:, c0:c0 + GD], pvB[:])

                    # Attention, n in two 512-halves to bound PSUM usage.
                    for nh in range(2):
                        ns = ts(nh, 512)
                        psO = [psAcc.tile([GD + 1, 512], F32, tag=f"psO{g}", name=f"psO{g}")
                               for g in range(2)]
                        sc = {}
                        for step in range(10):
                            if step < 8:
                                msl = ts(step, 128)
                                pss = [psS.tile([128, 512], F32,
                                                tag=f"pss{g}",
                                                name=f"pss{g}_{step}")
                                       for g in range(2)]
                                nc.tensor.matmul(
                                    pss[0][:], K2[0:64, msl], Q2[0:64, ns],
                                    start=True, stop=True,
                                    tile_position=(0, 0))
                                nc.tensor.matmul(
                                    pss[1][:], K2[64:128, msl],
                                    Q2[64:128, ns],
                                    start=True, stop=True,
                                    tile_position=(64, 0))
                                sc[step] = pss
                            if step >= 2:
                                mc = step - 2
                                for g in range(2):
                                    E = ep.tile([128, 512], BF16, tag=f"E{g}",
                                                name=f"E{g}_{mc}")
                                    nc.scalar.activation(
                                        E[:], sc[mc][g][:], EXP, scale=0.125)
                                    c0 = 65 * mc
                                    nc.tensor.matmul(
                                        psO[g][:], vts[g][:, c0:c0 + GD + 1],
                                        E[:], start=(mc == 0), stop=(mc == 7))
                        # stage numerators + denominators; normalize later
                        # (keeps the slow single-partition reciprocal off
                        # the PE critical path and frees psO banks early)
                        for g in range(2):
                            nc.vector.tensor_copy(
                                outN[p][GD * g:GD * (g + 1), ns],
                                psO[g][0:GD, :])
                            r = 32 * (2 * nh + g)
                            nc.vector.tensor_copy(
                                den4[r:r + 1, :], psO[g][GD:GD + 1, :])

                    # batched reciprocal now (DVE, overlaps next pair);
                    # broadcast+multiply deferred to batch end
                    rec4 = sm.tile([128, 512], F32, tag="rec4",
                                   name=f"rec4_{p}")
                    nc.vector.reciprocal(rec4[:], den4[:])
                    norm_jobs.append((p, rec4))

                for p, rec4 in norm_jobs:
                    for nh in range(2):
                        ns = ts(nh, 512)
                        psR = psS.tile([128, 512], F32, tag=f"pss{nh}",
                                       name=f"psR{nh}_{p}")
                        for g in range(2):
                            r = 32 * (2 * nh + g)
                            nc.tensor.matmul(
                                psR[GD * g:GD * (g + 1), :],
                                ones128[r:r + 1, 0:GD], rec4[r:r + 1, :],
                                start=True, stop=True,
                                tile_position=(r, GD * g))
                            rows = outN[p][GD * g:GD * (g + 1), ns]
                            nc.vector.tensor_mul(
                                rows, rows, psR[GD * g:GD * (g + 1), :])

                # out_proj: y[b] = woT.T @ outN (contraction over C=256)
                for ec in range(2):
                    yt = yp.tile([128, NT], F32, tag="yt")
                    for nh in range(2):
                        s = ts(nh, 512)
                        psY = psS.tile([128, 512], F32, tag=f"pss{nh}",
                                       name=f"psY{nh}")
                        for kc in range(2):
                            nc.tensor.matmul(
                                psY[:], wo_sb[kc][:, ts(ec, 128)],
                                outN[kc][:, s],
                                start=(kc == 0), stop=(kc == 1))
                        nc.vector.tensor_copy(yt[:, s], psY[:])
                    nc.sync.dma_start(y[b][ts(ec, 128), :], yt[:])

    nc.finalize()
    return nc


_NC_CACHE = None


def _get_nc():
    global _NC_CACHE
    if _NC_CACHE is None:
        _NC_CACHE = _build_program()
    return _NC_CACHE


def _prep_inputs(x, wq, wk, wv, wo):
    B = x.shape[0]
    xr = np.ascontiguousarray(x.reshape(B, 2, 128, NT), dtype=np.float32)
    # [G, d, c] -> [G, c, d] -> [pair, 128, d]
    wqT = np.ascontiguousarray(
        wq.transpose(0, 2, 1).reshape(2, 128, GD), dtype=np.float32)
    wkT = np.ascontiguousarray(
        wk.transpose(0, 2, 1).reshape(2, 128, GD), dtype=np.float32)
    wvT = np.ascontiguousarray(
        wv.transpose(0, 2, 1).reshape(2, 128, GD), dtype=np.float32)
    woT = np.ascontiguousarray(wo.T.reshape(2, 128, 256), dtype=np.float32)
    return xr, wqT, wkT, wvT, woT


def run(x, wq, wk, wv, wo, trace=False, **trace_kwargs):
    x = np.asarray(x, dtype=np.float32)
    B, C, H, W = x.shape
    xr, wqT, wkT, wvT, woT = _prep_inputs(
        x, np.asarray(wq, np.float32), np.asarray(wk, np.float32),
        np.asarray(wv, np.float32), np.asarray(wo, np.float32))
    in_maps = []
    for c in range(N_CORES):
        in_maps.append({
            "xs": xr[c * B_PER_CORE:(c + 1) * B_PER_CORE],
            "wqT": wqT, "wkT": wkT, "wvT": wvT, "woT": woT,
        })
    res = run_bass_kernel_spmd(_get_nc(), in_maps, list(range(N_CORES)),
                               trace=trace, **trace_kwargs)
    outs = [res.results[c]["y"] for c in range(N_CORES)]
    yfull = np.concatenate(outs, axis=0).reshape(B, C, H, W)
    return yfull.astype(np.float32), res


def kernel(x, wq, wk, wv, wo):
    out, _ = run(x, wq, wk, wv, wo, trace=False)
    return out



# revision 11
# speedup vs baseline: 1.2339x; 1.2339x over previous
"""GroupMixAttention Trainium2 kernel (8-core SPMD, batch-parallel), v2.

Problem: x[16,256,32,32]; per group g (4 groups of 64 ch):
  Q/K/V = wq/wk/wv[g] @ xg   (xg = [64, 1024])
  scores = (Q^T K)/8 ; attn = softmax(scores, -1) ; out = V @ attn^T
then y = wo @ concat(out).

Sharding: data-parallel over batch, 2 batches per core, no collectives.

v2 design notes (ACT-engine-bound at ~73us of exp):
  - All matmuls fp16 (1 cyc/row vs fp32's 4): x is cast to fp16 on host.
  - Q/K fold: scoresT[m,n] = sum_c U[c,m] x[c,n] with U = (wq^T wk) x;
    host sends wuT = wk^T wq as the lhsT for the U projection. One
    fp16 projection replaces both Q and K projections.
  - Scores tiles S_g [128(m-chunk), 1024(n)] span 2 PSUM banks; one
    1024-wide exp per (g, mc) minimizes ACT instruction overhead.
    S_g0/S_g1 alternate, acting as the double buffer so ACT never waits.
  - PV: psO_g[0:65, n] += VT_chunk^T @ E (lhsT = VT chunk [128, 65]
    with a ones column -> row 64 accumulates the softmax denominator).
    One open PSUM accumulation group per bank (hardware constraint).
  - Normalize: reciprocal_approx_fast on the single-partition den row,
    then one tensor_tensor mult per (p,g) with the reciprocal row
    partition-broadcast via broadcast_to — fused with the PSUM->SBUF
    eviction and the fp16 downcast. No PE broadcast matmuls.
  - PSUM: S0+S1 (4 banks) + O0+O1 (4 banks); U/VT prep and the
    out_proj accumulators reuse the O slots between attention loops.
"""

import os
import sys

import numpy as np

for _p in ("/opt/trn_rl_repo", "/root/.axon_site/_ro/trn_rl_repo"):
    if os.path.isdir(_p) and _p not in sys.path:
        sys.path.insert(0, _p)

import concourse.bass as bass
import concourse.mybir as mybir
import concourse.tile as tile
from concourse import bacc
from concourse.bass_utils import run_bass_kernel_spmd

F32 = mybir.dt.float32
F32R = mybir.dt.float32r
F16 = mybir.dt.float16
EXP = mybir.ActivationFunctionType.Exp
N_CORES = 8
B_PER_CORE = 2  # 16 batches / 8 cores
NT = 1024  # H*W
GD = 64    # group dim
ts = bass.ts


def _build_program():
    nc = bacc.Bacc("TRN2", target_bir_lowering=False, debug=False,
                   num_devices=N_CORES)
    xs = nc.dram_tensor("xs", [B_PER_CORE, 2, 128, NT], F16,
                        kind="ExternalInput").ap()
    wuT = nc.dram_tensor("wuT", [2, 128, GD], F16, kind="ExternalInput").ap()
    wvT = nc.dram_tensor("wvT", [2, 128, GD], F16, kind="ExternalInput").ap()
    woT = nc.dram_tensor("woT", [2, 128, 256], F16, kind="ExternalInput").ap()
    y = nc.dram_tensor("y", [B_PER_CORE, 256, NT], F32,
                       kind="ExternalOutput").ap()

    BP = B_PER_CORE

    with tile.TileContext(nc) as tc:
        from contextlib import ExitStack
        with ExitStack() as ctx:
            const = ctx.enter_context(tc.tile_pool(name="const", bufs=1))
            up = ctx.enter_context(tc.tile_pool(name="up", bufs=1))
            ep = ctx.enter_context(tc.tile_pool(name="ep", bufs=3))
            op = ctx.enter_context(tc.tile_pool(name="op", bufs=2))
            sp = ctx.enter_context(tc.tile_pool(name="sp", bufs=2))
            psS = ctx.enter_context(
                tc.tile_pool(name="psS", bufs=1, space="PSUM"))
            psO = ctx.enter_context(
                tc.tile_pool(name="psO", bufs=1, space="PSUM"))

            # ---- constants ----------------------------------------------
            wu_sb, wv_sb, wo_sb = [], [], []
            for p in range(2):
                t = const.tile([128, GD], F16, tag=f"wu{p}", name=f"wu{p}")
                nc.sync.dma_start(t[:], wuT[p])
                wu_sb.append(t)
                t = const.tile([128, GD], F16, tag=f"wv{p}", name=f"wv{p}")
                nc.sync.dma_start(t[:], wvT[p])
                wv_sb.append(t)
            for k in range(2):
                t = const.tile([128, 256], F16, tag=f"wo{k}", name=f"wo{k}")
                nc.sync.dma_start(t[:], woT[k])
                wo_sb.append(t)
            ones_sb = const.tile([128, GD], F32, tag="ones", name="ones")
            nc.gpsimd.memset(ones_sb[:], 1.0)

            # ---- x loads (host already cast to fp16) --------------------
            xh = {}
            for b in range(BP):
                for p in range(2):
                    t = const.tile([128, NT], F16, tag=f"xh{b}{p}",
                                   name=f"xh{b}{p}")
                    nc.sync.dma_start(t[:], xs[b, p])
                    xh[b, p] = t

            # ---- prep: U projection + V^T for one (b, p) ----------------
            Uh = {}
            VT = {}

            def prep(b, p):
                x2 = xh[b, p]
                # U = (wk^T wq) @ x, both groups packed diagonally.
                psU = psO.tile([128, NT], F32, tag="O0", name=f"psU{b}{p}")
                for g in range(2):
                    r = slice(64 * g, 64 * (g + 1))
                    for nh in range(2):
                        s = ts(nh, 512)
                        nc.tensor.matmul(
                            psU[r, s], wu_sb[p][r, :], x2[r, s],
                            start=True, stop=True,
                            tile_position=(64 * g, 64 * g))
                u = up.tile([128, NT], F16, tag=f"Uh{b}{p}", name=f"Uh{b}{p}")
                nc.vector.tensor_copy(u[:], psU[:])
                Uh[b, p] = u

                # V^T chunks [m(128), d(64)] for both groups.
                psV = psO.tile([128, 16, GD], F32, tag="O1", name=f"psV{b}{p}")
                for g in range(2):
                    r = slice(64 * g, 64 * (g + 1))
                    for mc in range(8):
                        nc.tensor.matmul(
                            psV[:, 8 * g + mc, :], x2[r, ts(mc, 128)],
                            wv_sb[p][r, :],
                            start=True, stop=True, tile_position=(64 * g, 0))
                for g in range(2):
                    vt = up.tile([128, 8, GD + 1], F16, tag=f"VT{b}{p}{g}",
                                 name=f"VT{b}{p}{g}")
                    nc.vector.memset(vt[:, :, GD:GD + 1], 1.0)
                    nc.vector.tensor_copy(
                        vt[:, :, 0:GD], psV[:, 8 * g:8 * (g + 1), :])
                    VT[b, p, g] = vt

            # ---- attention loop for one (b, p) --------------------------
            PSO = {}

            def attn(b, p):
                x2 = xh[b, p]
                u = Uh[b, p]
                pso = [psO.tile([128, NT], F32, tag=f"O{g}",
                                name=f"psO{b}{p}{g}") for g in range(2)]
                PSO[b, p] = pso
                E = {}
                for step in range(9):
                    if step < 8:
                        mc = step
                        for g in range(2):
                            r = slice(64 * g, 64 * (g + 1))
                            S = psS.tile([128, NT], F32, tag=f"S{g}",
                                         name=f"S{b}{p}{g}_{mc}")
                            for nh in range(2):
                                s = ts(nh, 512)
                                nc.tensor.matmul(
                                    S[:, s], u[r, ts(mc, 128)], x2[r, s],
                                    start=True, stop=True,
                                    tile_position=(64 * g, 0))
                            e = ep.tile([128, NT], F16, tag=f"E{g}",
                                        name=f"E{b}{p}{g}_{mc}")
                            nc.scalar.activation(e[:], S[:], EXP, scale=0.125)
                            E[g] = e
                    if step >= 1:
                        mc = step - 1
                        for g in range(2):
                            for nh in range(2):
                                s = ts(nh, 512)
                                nc.tensor.matmul(
                                    pso[g][0:GD + 1, s],
                                    VT[b, p, g][:, mc, :], E[g + 2][:, s],
                                    start=(mc == 0), stop=(mc == 7))
                    # rotate: PV at step reads E issued at step-1
                    for g in range(2):
                        if g in E:
                            E[g + 2] = E.pop(g)

            # ---- normalize + evict for one (b, p) -----------------------
            out16 = {}

            def norm(b, p):
                pso = PSO.pop((b, p))
                o = op.tile([128, NT], F16, tag=f"o16_{p}", name=f"o16_{b}{p}")
                out16[b, p] = o
                for g in range(2):
                    rec = sp.tile([GD + 1, NT], F32, tag="rec",
                                  name=f"rec{b}{p}{g}")
                    nc.vector.reciprocal(
                        rec[GD:GD + 1, :], pso[g][GD:GD + 1, :])
                    # K=1 matmul broadcasts rec into the unused
                    # partitions 64:128 of the pso banks.
                    for nh in range(2):
                        s = ts(nh, 512)
                        nc.tensor.matmul(
                            pso[g][GD:GD + 64, s],
                            ones_sb[GD:GD + 1, :],
                            rec[GD:GD + 1, s],
                            start=True, stop=True, tile_position=(64, 64))
                    # TT may read only one PSUM operand: stage the
                    # broadcast reciprocal rows in SBUF first.
                    recB = sp.tile([GD, NT], F32, tag="recB",
                                   name=f"recB{b}{p}{g}")
                    nc.vector.tensor_copy(recB[:], pso[g][GD:GD + 64, :])
                    nc.vector.tensor_tensor(
                        out=o[64 * g:64 * (g + 1), :],
                        in0=pso[g][0:GD, :],
                        in1=recB[:],
                        op=mybir.AluOpType.mult)

            # ---- tail: out_proj + store ---------------------------------
            def tail(b):
                for ec in range(2):
                    psY = psO.tile([128, NT], F32, tag=f"O{ec}",
                                   name=f"psY{b}{ec}")
                    for nh in range(2):
                        s = ts(nh, 512)
                        for kc in range(2):
                            nc.tensor.matmul(
                                psY[:, s], wo_sb[kc][:, ts(ec, 128)],
                                out16[b, kc][:, s],
                                start=(kc == 0), stop=(kc == 1))
                    ysb = sp.tile([128, NT], F32, tag="ysb",
                                  name=f"ysb{b}{ec}")
                    nc.vector.tensor_copy(ysb[:], psY[:])
                    nc.sync.dma_start(y[b][ts(ec, 128), :], ysb[:])

            # ---- schedule -----------------------------------------------
            prep(0, 0)
            prep(0, 1)
            attn(0, 0)
            norm(0, 0)
            prep(1, 0)
            attn(0, 1)
            norm(0, 1)
            prep(1, 1)
            attn(1, 0)
            norm(1, 0)
            tail(0)
            attn(1, 1)
            norm(1, 1)
            tail(1)

    nc.finalize()
    return nc


_NC_CACHE = None


def _get_nc():
    global _NC_CACHE
    if _NC_CACHE is None:
        _NC_CACHE = _build_program()
    return _NC_CACHE


def _prep_inputs(x, wq, wk, wv, wo):
    B = x.shape[0]
    xr = np.ascontiguousarray(x.reshape(B, 2, 128, NT), dtype=np.float16)
    # U-projection lhsT per group: wuT_g = wk_g^T @ wq_g  [c, c']
    wu = np.einsum('gdc,gde->gce', wk.astype(np.float64),
                   wq.astype(np.float64))
    wuT = np.ascontiguousarray(wu.reshape(2, 128, GD), dtype=np.float16)
    # V^T rhs: [G, d, c] -> [G, c, d] -> [pair, 128, d]
    wvT = np.ascontiguousarray(
        wv.transpose(0, 2, 1).reshape(2, 128, GD), dtype=np.float16)
    woT = np.ascontiguousarray(wo.T.reshape(2, 128, 256), dtype=np.float16)
    return xr, wuT, wvT, woT


def run(x, wq, wk, wv, wo, trace=False, **trace_kwargs):
    x = np.asarray(x, dtype=np.float32)
    B, C, H, W = x.shape
    xr, wuT, wvT, woT = _prep_inputs(
        x, np.asarray(wq, np.float32), np.asarray(wk, np.float32),
        np.asarray(wv, np.float32), np.asarray(wo, np.float32))
    in_maps = []
    for c in range(N_CORES):
        in_maps.append({
            "xs": xr[c * B_PER_CORE:(c + 1) * B_PER_CORE],
            "wuT": wuT, "wvT": wvT, "woT": woT,
        })
    res = run_bass_kernel_spmd(_get_nc(), in_maps, list(range(N_CORES)),
                               trace=trace, **trace_kwargs)
    outs = [res.results[c]["y"] for c in range(N_CORES)]
    yfull = np.concatenate(outs, axis=0).reshape(B, C, H, W)
    return yfull.astype(np.float32), res


def kernel(x, wq, wk, wv, wo):
    out, _ = run(x, wq, wk, wv, wo, trace=False)
    return out
